# revision 9
# baseline (speedup 1.0000x reference)
"""Trainium2 Bass kernel for nn_ComplexMixture.

Per batch element b (R = input_real[b] [S,D], I = input_imag[b] [S,D], w [S]):
    out_r = (w*R)^T R + (w*I)^T I        (symmetric)
    out_i = (w*I)^T R - (w*R)^T I        (antisymmetric)

With A = sqrt(w)*R, B = sqrt(w)*I (w >= 0):
    N  = B^T A            (full 768x768, 48 matmuls)
    G  = (A+B)^T (A+B)    (upper 9 of 12 [128,384] pair-tiles, 36 matmuls)
    out_r = G - N - N^T   (symmetric   -> upper tiles only)
    out_i = N - N^T       (antisymmetric -> upper tiles only)

This replaces the 4-term scheme (144 matmuls) with 84 matmuls plus 27 cheap
PE transposes ([128,128] fp16, ~1 cyc/row): N is evacuated to fp16 SBUF
(pN, ScalarE copies) and each pair-tile needs G (fp32 PSUM bank, 4 matmuls)
plus N^T (fp16 PSUM bank psnt, 3 transposes of pN).  Evac is fused and
spread across engines (GpSimd can neither read PSUM nor run
TensorScalarPtr, so its combines are SBUF-only tensor_tensor subtracts):
    DVE : tmp = psG - pN   (= G - N)      ACT : nt = copy(psnt)  (= N^T)
    Pool: or  = tmp - nt                  DVE : oi = -psnt + pN  (2x fp16)
    Pool: oin = nt - pN    (= -oi, device-negated for the host mirror)

Outputs are stored fp16 (host upcasts to fp32: pure dtype cast), halving
store traffic vs fp32.  out_r's strictly-lower [384,384] block and out_i's
(from the device-negated oin tile) are mirrored on host by pure transpose
copies, exactly like the previous kernel.

Sharding: data-parallel over batch, one batch element per NeuronCore (B == 8
== n_cores).  Host marshalling: R/I cast to fp16, sqrt(w) precomputed,
plus a tiny fp16 identity for the PE transposes.

N-phase ordering is k-outer over 3-row groups so the PE consumes input
chunks no faster than the DMA intake delivers them; a short burst of dummy
matmuls bridges the PE from the preamble into the first real matmul so the
p-state ramp starts early.
"""

import sys
import types

import numpy as np

# If the environment requests tracing (BASS_TRACE=1) but the image lacks
# antenv.axon_hooks, bass_utils would crash importing it; provide a no-op
# hook registry so tracing degrades gracefully instead.
try:
    import antenv.axon_hooks  # noqa: F401
except ImportError:
    _hooks = types.ModuleType("antenv.axon_hooks")
    _hooks._hook = None
    _hooks.set_axon_ntff_profile_hook = lambda h: setattr(_hooks, "_hook", h)
    _hooks.get_axon_ntff_profile_hook = lambda: _hooks._hook
    sys.modules["antenv.axon_hooks"] = _hooks

import concourse.bacc as bacc
import concourse.bass_utils as bass_utils
import concourse.mybir as mybir
import concourse.tile as tile

B, S, D = 8, 512, 768
P = 128          # SBUF/PSUM partitions; matmul contraction tile
KC = S // P      # 4 contraction chunks per operand
MT = D // P      # 6 output row tiles
NW = 384         # matmul moving free dim (<=512 fp32 PSUM bank)
NB = D // NW     # 2 output column blocks
N_CORES = 8
N_PREWARM = 8    # dummy 128-col matmuls bridging preamble -> first real mm

# 9 computed (m, n) pair-tiles: full upper at [384] granularity, row-major
# so both halves of an output row complete adjacently (merged row stores)
PAIRS = [(0, 0), (0, 1), (1, 0), (1, 1), (2, 0), (2, 1), (3, 1), (4, 1), (5, 1)]

_CACHE: dict = {}


def _build():
    f32, f16 = mybir.dt.float32, mybir.dt.float16
    add_op = mybir.AluOpType.add
    sub_op = mybir.AluOpType.subtract
    mul_op = mybir.AluOpType.mult
    nc = bacc.Bacc(
        "TRN2", target_bir_lowering=False, debug=False, num_devices=N_CORES
    )
    # Host-packed partition-major: r_in[p, k*D:(k+1)*D] = R[k*P+p, :], so a
    # whole k-chunk group is one DMA with long per-partition descriptors.
    r_d = nc.dram_tensor("r_in", [P, KC * D], f16, kind="ExternalInput").ap()
    i_d = nc.dram_tensor("i_in", [P, KC * D], f16, kind="ExternalInput").ap()
    s_d = nc.dram_tensor("s_in", [P, KC], f32, kind="ExternalInput").ap()
    e_d = nc.dram_tensor("e_in", [P, P], f16, kind="ExternalInput").ap()
    or_d = nc.dram_tensor("or_out", [D, D], f16, kind="ExternalOutput").ap()
    oi_d = nc.dram_tensor("oi_out", [D, D], f16, kind="ExternalOutput").ap()
    # negated upper-right block of out_i; host transposes it into the
    # skipped lower-left block (out_i is antisymmetric)
    oin_d = nc.dram_tensor("oin_out", [D // 2, NW], f16, kind="ExternalOutput").ap()

    def ms(m):
        return slice(m * P, (m + 1) * P)

    def nsl(n):
        return slice(n * NW, (n + 1) * NW)

    with tile.TileContext(nc) as tc:
        with (
            tc.tile_pool(name="const", bufs=1) as cpool,
            tc.tile_pool(name="stage", bufs=1) as spool,
            tc.tile_pool(name="abc", bufs=1) as apool,
            tc.tile_pool(name="nsb", bufs=1) as npool,
            tc.tile_pool(name="osb", bufs=2) as opool,
            tc.tile_pool(name="ps", bufs=1, space="PSUM") as pspool,
        ):
            # Small constants ride the otherwise-idle gpsimd ring.
            s_t = cpool.tile([P, KC], f32, name="s_t")
            nc.gpsimd.dma_start(s_t[:], s_d)
            idn = cpool.tile([P, P], f16, name="idn")
            nc.gpsimd.dma_start(idn[:], e_d)

            # PE prewarm on zeroed tiles: releases the p-state ramp before
            # the first real matmul.  Uses the ps0 PSUM tag (released well
            # before the first pair needs it again).
            zw = cpool.tile([P, 2 * P], f16, name="zw")
            nc.vector.memset(zw[:], 0.0)
            pz = pspool.tile([P, NW], f32, name="pz", tag="ps0")
            for _ in range(N_PREWARM):
                nc.tensor.matmul(
                    pz[:, 0:P], zw[:, 0:P], zw[:, P : 2 * P], start=True, stop=True
                )

            # Inputs staggered in consumption order; r on the sync HWDGE
            # ring, i on the scalar ring, late bulk pairs on gpsimd SWDGE.
            r0 = spool.tile([P, D], f16, name="r0", tag="r0")
            i0 = spool.tile([P, D], f16, name="i0", tag="i0")
            r1 = spool.tile([P, D], f16, name="r1", tag="r1")
            i1 = spool.tile([P, D], f16, name="i1", tag="i1")
            r23 = spool.tile([P, 2 * D], f16, name="r23", tag="r23")
            i23 = spool.tile([P, 2 * D], f16, name="i23", tag="i23")
            nc.sync.dma_start(r0[:], r_d[:, 0:D])
            nc.scalar.dma_start(i0[:], i_d[:, 0:D])
            nc.sync.dma_start(r1[:], r_d[:, D : 2 * D])
            nc.scalar.dma_start(i1[:], i_d[:, D : 2 * D])
            nc.gpsimd.dma_start(r23[:], r_d[:, 2 * D : 4 * D])
            nc.gpsimd.dma_start(i23[:], i_d[:, 2 * D : 4 * D])

            def rfk(k):
                return (r0[:], r1[:], r23[:, 0:D], r23[:, D : 2 * D])[k]

            def ifk(k):
                return (i0[:], i1[:], i23[:, 0:D], i23[:, D : 2 * D])[k]

            # Per-row scaling on DVE: A = sw*R, B = sw*I (fp16).  A/B first
            # (tight deadlines: the N matmuls), AB = A+B after (needed only
            # by the pair phase).
            At, Bt, ABt = [], [], []
            for k in range(KC):
                a = apool.tile([P, D], f16, name=f"A{k}", tag=f"A{k}")
                nc.vector.tensor_scalar_mul(a[:], rfk(k), s_t[:, k : k + 1])
                b = apool.tile([P, D], f16, name=f"B{k}", tag=f"B{k}")
                nc.vector.tensor_scalar_mul(b[:], ifk(k), s_t[:, k : k + 1])
                At.append(a)
                Bt.append(b)
            for k in range(KC):
                ab = apool.tile([P, D], f16, name=f"AB{k}", tag=f"AB{k}")
                nc.vector.tensor_tensor(ab[:], At[k][:], Bt[k][:], add_op)
                ABt.append(ab)

            # pN[m] = +N row-block m, fp16 in SBUF (ScalarE evac copy).
            pn = [
                npool.tile([P, D], f16, name=f"pn{m}", tag=f"pn{m}")
                for m in range(MT)
            ]

            # N = B^T A, computed full, in two 3-row groups with k outermost
            # so PE chunk consumption tracks DMA arrival order.
            for g in (0, 3):
                psn = [
                    pspool.tile([P, NW], f32, name=f"psn{g}_{j}", tag=f"ps{j}")
                    for j in range(6)
                ]
                for k in range(KC):
                    for mm in range(3):
                        m = g + mm
                        for h in range(NB):
                            nc.tensor.matmul(
                                psn[2 * mm + h][:],
                                Bt[k][:, ms(m)],
                                At[k][:, nsl(h)],
                                start=(k == 0),
                                stop=(k == KC - 1),
                            )
                for mm in range(3):
                    m = g + mm
                    for h in range(NB):
                        nc.scalar.copy(pn[m][:, nsl(h)], psn[2 * mm + h][:])

            # Pair phase: per (m,n) tile, psG accumulates G (fp32 bank) and
            # psnt takes +N^T via three fp16 PE transposes of pN (fp16 PSUM
            # tile).  GpSimd can neither read PSUM nor run TensorScalarPtr,
            # so its combines are plain SBUF tensor_tensor subtracts:
            #   DVE : tmp = psG - pN        (= G - N, fp16 SBUF, stt)
            #   ACT : nt  = copy(psnt)      (= N^T, fp16 SBUF)
            #   Pool: or  = tmp - nt        (= G - N - N^T, TT sub)
            #   DVE : oi  = -psnt + pN      (= N - N^T, stt, 2x fp16)
            #   Pool: oin = nt - pN         (= N^T - N = -oi, TT sub)
            # or/oi halves land in per-row staging tiles -> one store per row.
            or_row = oi_row = None
            for j, (m, n) in enumerate(PAIRS):
                psg = pspool.tile(
                    [P, NW], f32, name=f"psg{j}", tag=f"ps{(2 * j) % 6}"
                )
                psnt = pspool.tile(
                    [P, NW], f16, name=f"psnt{j}", tag=f"ps{(2 * j + 1) % 6}"
                )
                for k in range(KC):
                    nc.tensor.matmul(
                        psg[:], ABt[k][:, ms(m)], ABt[k][:, nsl(n)],
                        start=(k == 0), stop=(k == KC - 1),
                    )
                for c in range(3):
                    cc = 3 * n + c
                    nc.tensor.matmul(
                        psnt[:, c * P : (c + 1) * P],
                        pn[cc][:, ms(m)],
                        idn[:],
                        is_transpose=True,
                        start=(c == 0),
                        stop=(c == 2),
                    )

                first_in_row = or_row is None
                if first_in_row:
                    or_row = opool.tile([P, D], f16, name=f"or_row{m}", tag="or_row")
                    oi_row = opool.tile([P, D], f16, name=f"oi_row{m}", tag="oi_row")

                tmp = opool.tile([P, NW], f16, name=f"tmp{j}", tag="tmp")
                # tmp = G - N
                nc.vector.scalar_tensor_tensor(
                    tmp[:], psg[:], 1.0, pn[m][:, nsl(n)], mul_op, sub_op
                )
                nt = opool.tile([P, NW], f16, name=f"nt{j}", tag="nt")
                nc.scalar.copy(nt[:], psnt[:])
                # out_r = (G - N) - N^T
                nc.gpsimd.tensor_tensor(
                    or_row[:, nsl(n)], tmp[:], nt[:], sub_op
                )
                # out_i = -(N^T) + N
                nc.vector.scalar_tensor_tensor(
                    oi_row[:, nsl(n)], psnt[:], -1.0, pn[m][:, nsl(n)], mul_op, add_op
                )

                if n == 1 and m < 3:
                    oin_sb = opool.tile(
                        [P, NW], f16, name=f"oin_sb{j}", tag="oin_sb"
                    )
                    # -out_i = N^T - N  (device-negated for the host mirror)
                    nc.gpsimd.tensor_tensor(
                        oin_sb[:], nt[:], pn[m][:, nsl(n)], sub_op
                    )
                    nc.gpsimd.dma_start(oin_d[ms(m), :], oin_sb[:])

                if n == 1:  # every row's final pair has n == 1
                    cs = slice(0, D) if m < 3 else nsl(1)
                    nc.sync.dma_start(or_d[ms(m), cs], or_row[:, cs])
                    nc.scalar.dma_start(oi_d[ms(m), cs], oi_row[:, cs])
                    or_row = oi_row = None

    nc.compile()
    return nc


def get_nc():
    if "nc" not in _CACHE:
        _CACHE["nc"] = _build()
    return _CACHE["nc"]


def make_in_maps(input_real, input_imag, weight):
    input_real = np.asarray(input_real)
    input_imag = np.asarray(input_imag)
    weight = np.asarray(weight, dtype=np.float32)
    # pack [S, D] -> [P, KC*D]: row p holds chunks k=0..KC-1 concatenated
    r16 = (
        input_real.astype(np.float16)
        .reshape(B, KC, P, D)
        .transpose(0, 2, 1, 3)
        .reshape(B, P, KC * D)
    )
    i16 = (
        input_imag.astype(np.float16)
        .reshape(B, KC, P, D)
        .transpose(0, 2, 1, 3)
        .reshape(B, P, KC * D)
    )
    sq = np.sqrt(weight).astype(np.float32)  # [B, S]
    s_pack = sq.reshape(B, KC, P).transpose(0, 2, 1)  # [B, P, KC]
    eye = np.ascontiguousarray(np.eye(P, dtype=np.float16))
    return [
        {
            "r_in": np.ascontiguousarray(r16[b]),
            "i_in": np.ascontiguousarray(i16[b]),
            "s_in": np.ascontiguousarray(s_pack[b]),
            "e_in": eye,
        }
        for b in range(B)
    ]


def unshard_one(res: dict) -> tuple[np.ndarray, np.ndarray]:
    """Device outputs (fp16, upper tiles) -> full fp32 (out_r, out_i)."""
    out_r = np.asarray(res["or_out"]).astype(np.float32)
    out_i = np.asarray(res["oi_out"]).astype(np.float32)
    # Mirror the device-skipped strictly-lower blocks (pure transpose
    # copies): out_r is symmetric; out_i's mirror block was negated on
    # device into oin_out.
    out_r[NW:D, 0:NW] = out_r[0:NW, NW:D].T
    out_i[NW:D, 0:NW] = np.asarray(res["oin_out"]).astype(np.float32).T
    return out_r, out_i


def run(input_real, input_imag, weight, **spmd_kwargs):
    nc = get_nc()
    res = bass_utils.run_bass_kernel_spmd(
        nc,
        make_in_maps(input_real, input_imag, weight),
        core_ids=list(range(N_CORES)),
        **spmd_kwargs,
    )
    outs = [unshard_one(res.results[b]) for b in range(B)]
    out_r = np.stack([o[0] for o in outs])
    out_i = np.stack([o[1] for o in outs])
    return (out_r, out_i), res


def kernel(input_real, input_imag, weight):
    (out_r, out_i), _ = run(input_real, input_imag, weight)
    return (out_r, out_i)
